# revision 13
# baseline (speedup 1.0000x reference)
"""Longformer sliding-window self-attention (MBart variant) on 8 TRN2 cores.

Sequence-parallel sharding: core c -> batch c//4, queries
[1024*(c%4), 1024*(c%4+1)). Each core gets a halo'd, transposed,
bf16 slice of hidden states (1536 rows), computes Q/K/V projections,
banded attention, and the output projection, returning a [768, 1024]
fp32 transposed output slice. Host re-assembles [2, 4096, 768].

Design notes (all rewrites exact up to bf16 rounding of the inputs,
measured ~4e-3 max-rel on the fixed problem seed vs the 2e-2 gate):
  - hidden states + weights ship as bf16 (halves HBM traffic); query
    scale folded into Wq/bq, Wk bias cancels in softmax, Wv bias
    commutes through the probs (bo_eff = bo + Wo @ bv).
  - banded scores per (chunk n, head h) are trimmed: key tile 0 only
    covers queries 0:128, tile 5 only 128:256 (rest is out of band) ->
    1280 score columns in three [128,<=512] PSUM tiles.
  - band + sequence-edge masking is a post-exp multiply by a 0/1 bf16
    tile (DVE 2x mode) instead of a -1e9 pre-exp add; a non-zero
    attention_mask (general path) adds its per-key bias pre-exp via
    tensor_scalar (bias is constant along queries = per-partition).
  - context matmuls run TRANSPOSED: lhsT = exp tile [keys, queries],
    rhs = V augmented with a ones column [keys, 65] -> PSUM [queries,
    64 ctx + denominator]. Queries on partitions make the softmax
    denominator a per-partition scalar: one reciprocal [128,1] + one
    tensor_scalar_mul per (n, h, q-half). 128x128 bf16 DMA transposes
    rebuild the [D, seq] layout for the output projection.
  - emission order software-pipelines the whole kernel: V proj, then
    per head-pair j: Q(j), K(j), the 8 attention units (skewed so PE
    streams unit u+1 scores while unit u waits on exp), then that
    pair's DMA transposes. Keeps PE dense (HAM stays warm) and starts
    ScalarE exp work ~50 us earlier.
"""

import numpy as np
import ml_dtypes

# problem shapes (fixed by the task)
B, S, D, H = 2, 4096, 768, 12
DH = D // H            # 64
W = 256                # one-sided window == chunk size b
NEG = -1e9
NCORES = 8
G = 4                  # sequence groups per batch (8 cores / 2 batches)
SLOC = S // G          # 1024 queries per core
SH = SLOC + 2 * W      # 1536 halo'd rows per core
NB = SLOC // W         # 4 chunks per core
P = 128
DJ = D // P            # 6 tiles of 128 over the model dim

_PROGRAM_CACHE: dict = {}

# ctx matmul source columns in the expt slab per q-half: (key tile t, col)
# slab cols: t0(q0:128)->0:128, t1->128:384, t2->384:640, t3->640:896,
#            t4->896:1152, t5(q128:256)->1152:1280
_CTX_QH = [
    [(0, 0), (1, 128), (2, 384), (3, 640), (4, 896)],          # q 0:128
    [(1, 256), (2, 512), (3, 768), (4, 1024), (5, 1152)],      # q 128:256
]


def _build_program(general_mask: bool):
    import concourse.bacc as bacc
    import concourse.mybir as mybir
    import concourse.tile as tile
    from contextlib import ExitStack

    F32 = mybir.dt.float32
    BF = mybir.dt.bfloat16
    AF = mybir.ActivationFunctionType

    nc = bacc.Bacc("TRN2", target_bir_lowering=False, debug=False)

    hsT = nc.dram_tensor("hsT", [D, SH], BF, kind="ExternalInput")
    wqT = nc.dram_tensor("wqT", [D, D], BF, kind="ExternalInput")
    wkT = nc.dram_tensor("wkT", [D, D], BF, kind="ExternalInput")
    wvT = nc.dram_tensor("wvT", [D, D], BF, kind="ExternalInput")
    woT = nc.dram_tensor("woT", [D, D], BF, kind="ExternalInput")
    bq = nc.dram_tensor("bq", [P, DJ], F32, kind="ExternalInput")
    boe = nc.dram_tensor("boe", [P, DJ], F32, kind="ExternalInput")
    masks = nc.dram_tensor("masks", [NB, 2, P, 384], BF, kind="ExternalInput")
    if general_mask:
        gbias = nc.dram_tensor("gbias", [NB, P, 6], F32, kind="ExternalInput")
    outT = nc.dram_tensor("outT", [D, SLOC], F32, kind="ExternalOutput")

    with tile.TileContext(nc) as tc, ExitStack() as stack:
        const = stack.enter_context(tc.tile_pool(name="const", bufs=1))
        qt_p = stack.enter_context(tc.tile_pool(name="qt", bufs=1))
        kt_p = stack.enter_context(tc.tile_pool(name="kt", bufs=1))
        ct_p = stack.enter_context(tc.tile_pool(name="ct", bufs=1))
        va_p = stack.enter_context(tc.tile_pool(name="va", bufs=1))
        w_p = stack.enter_context(tc.tile_pool(name="wts", bufs=1))
        oa_p = stack.enter_context(tc.tile_pool(name="oa", bufs=1))

        # ---- input DMAs, one per tensor, hidden states first ---------
        hs_stack = ExitStack()
        hs_p = hs_stack.enter_context(tc.tile_pool(name="hs", bufs=1))
        hs_all = hs_p.tile([P, DJ * SH], BF, tag="hs")
        hs_view = hs_all.rearrange("p (t c) -> p t c", c=SH)
        hsT_view = hsT.rearrange("(t p) c -> p t c", p=P)
        for ch in range(2):
            nc.sync.dma_start(
                out=hs_view[:, :, SH // 2 * ch: SH // 2 * (ch + 1)],
                in_=hsT_view[:, :, SH // 2 * ch: SH // 2 * (ch + 1)],
            )
        HS = [hs_all[:, SH * i: SH * (i + 1)] for i in range(DJ)]

        wv_all = w_p.tile([P, DJ * D], BF, tag="wv")
        wv_view = wv_all.rearrange("p (t c) -> p t c", c=D)
        wvT_view = wvT.rearrange("(t p) c -> p t c", p=P)
        for ch in range(2):
            nc.sync.dma_start(
                out=wv_view[:, :, 384 * ch: 384 * (ch + 1)],
                in_=wvT_view[:, :, 384 * ch: 384 * (ch + 1)],
            )
        WV = [wv_all[:, D * i: D * (i + 1)] for i in range(DJ)]
        wq_all = w_p.tile([P, DJ * D], BF, tag="wq")
        nc.sync.dma_start(
            out=wq_all.rearrange("p (t c) -> p t c", c=D),
            in_=wqT.rearrange("(t p) c -> p t c", p=P),
        )
        WQ = [wq_all[:, D * i: D * (i + 1)] for i in range(DJ)]
        wk_all = w_p.tile([P, DJ * D], BF, tag="wk")
        nc.sync.dma_start(
            out=wk_all.rearrange("p (t c) -> p t c", c=D),
            in_=wkT.rearrange("(t p) c -> p t c", p=P),
        )
        WK = [wk_all[:, D * i: D * (i + 1)] for i in range(DJ)]

        mask_sb = const.tile([P, NB * 2 * 384], BF, tag="masks")
        nc.sync.dma_start(
            out=mask_sb.rearrange("p (n s c) -> p n s c", s=2, c=384),
            in_=masks.rearrange("n s p c -> p n s c"),
        )
        bq_sb = const.tile([P, DJ], F32, tag="bq")
        nc.sync.dma_start(out=bq_sb[:], in_=bq[:, :])
        boe_sb = const.tile([P, DJ], F32, tag="boe")
        nc.sync.dma_start(out=boe_sb[:], in_=boe[:, :])
        if general_mask:
            gb_sb = const.tile([P, NB * 6], F32, tag="gbias")
            nc.sync.dma_start(
                out=gb_sb.rearrange("p (n c) -> p n c", c=6),
                in_=gbias.rearrange("n p c -> p n c"),
            )
        wo_all = w_p.tile([P, DJ * D], BF, tag="wo")
        nc.sync.dma_start(
            out=wo_all.rearrange("p (t c) -> p t c", c=D),
            in_=woT.rearrange("(t p) c -> p t c", p=P),
        )
        WO = [wo_all[:, D * i: D * (i + 1)] for i in range(DJ)]

        QT = [qt_p.tile([P, SLOC], BF, tag=f"qt{j}", name=f"qt{j}") for j in range(DJ)]
        KT = [kt_p.tile([P, SH], BF, tag=f"kt{j}", name=f"kt{j}") for j in range(DJ)]
        CT = [ct_p.tile([P, SLOC], BF, tag=f"ct{j}", name=f"ct{j}") for j in range(DJ)]
        VA = [va_p.tile([P, H * (DH + 1)], BF, tag=f"va{s}", name=f"va{s}")
              for s in range(SH // P)]
        for st in range(SH // P):
            nc.vector.memset(VA[st][:], 1.0)   # ones columns survive the V copy

        # ---------------- V projection --------------------------------
        with tc.tile_pool(name="ps2", bufs=4, space="PSUM") as ps2:
            for vh in range(2):
                for st in range(SH // P):
                    ps = ps2.tile([P, 384], F32, tag="ps2")
                    for i in range(DJ):
                        nc.tensor.matmul(
                            ps[:],
                            HS[i][:, P * st: P * (st + 1)],
                            WV[i][:, 384 * vh: 384 * (vh + 1)],
                            start=(i == 0),
                            stop=(i == DJ - 1),
                        )
                    view = VA[st].rearrange("p (h e) -> p h e", e=DH + 1)
                    nc.vector.tensor_copy(
                        view[:, 6 * vh: 6 * (vh + 1), 0:DH],
                        ps[:].rearrange("p (h e) -> p h e", e=DH),
                    )

        # ------- interleaved Q/K projections + banded attention -------
        with (
            tc.tile_pool(name="expp", bufs=3) as exp_p,
            tc.tile_pool(name="ctx", bufs=1) as ctx_p,
            tc.tile_pool(name="rcp", bufs=6) as rc_p,
            tc.tile_pool(name="pss", bufs=6, space="PSUM") as pss,
            tc.tile_pool(name="psc", bufs=2, space="PSUM") as psc,
        ):
            CX = [ctx_p.tile([P, H * DH], BF, tag=f"cx{u}", name=f"cx{u}")
                  for u in range(2 * NB)]

            def emit_scores(h, n):
                """Score matmuls + exp + band zeroing for one (head, chunk).
                Returns the bf16 exp slab [128 keys, 1280]."""
                jq, r0 = h // 2, DH * (h % 2)
                kb = W * n
                expt = exp_p.tile([P, 1280], BF, tag="e", name=f"e{n}_{h}")
                segs = [  # (group tiles, exp slab col, mask slot or None)
                    ([(0, 0, 128, 0), (1, 128, 384, 0)], 0, 0),
                    ([(2, 0, 256, 0), (3, 256, 512, 0)], 384, None),
                    ([(4, 0, 256, 0), (5, 256, 384, 128)], 896, 1),
                ]
                for gi, (tiles, ecol, mslot) in enumerate(segs):
                    gp = pss.tile([P, 512], F32, tag="s", name=f"s{gi}_{n}_{h}")
                    width = tiles[-1][2]
                    for t, c0, c1, qlo in tiles:
                        nc.tensor.matmul(
                            gp[:, c0:c1],
                            KT[jq][r0:r0 + DH, kb + P * t: kb + P * (t + 1)],
                            QT[jq][r0:r0 + DH, W * n + qlo: W * n + qlo + (c1 - c0)],
                            start=True,
                            stop=True,
                        )
                    if general_mask:
                        for t, c0, c1, qlo in tiles:
                            nc.vector.tensor_scalar_add(
                                gp[:, c0:c1], gp[:, c0:c1],
                                gb_sb[:, 6 * n + t: 6 * n + t + 1],
                            )
                    nc.scalar.activation(
                        expt[:, ecol: ecol + width], gp[:, 0:width], AF.Exp
                    )
                    if mslot is not None:
                        moff = (n * 2 + mslot) * 384
                        nc.vector.tensor_mul(
                            expt[:, ecol: ecol + 384],
                            expt[:, ecol: ecol + 384],
                            mask_sb[:, moff: moff + 384],
                        )
                return expt

            def emit_ctx(h, n, expt):
                """Transposed context + normalize into the CX slabs."""
                for qh in range(2):
                    cps = psc.tile([P, DH + 1], F32, tag="c", name=f"c{n}_{h}_{qh}")
                    segs = _CTX_QH[qh]
                    for si, (t, c0) in enumerate(segs):
                        nc.tensor.matmul(
                            cps[:],
                            expt[:, c0: c0 + P],
                            VA[2 * n + t][:, (DH + 1) * h: (DH + 1) * (h + 1)],
                            start=(si == 0),
                            stop=(si == len(segs) - 1),
                        )
                    rc = rc_p.tile([P, 1], F32, tag="rc", name=f"rc{n}_{h}_{qh}")
                    nc.vector.reciprocal(out=rc[:], in_=cps[:, DH: DH + 1])
                    nc.vector.tensor_scalar_mul(
                        CX[2 * n + qh][:, DH * h: DH * (h + 1)],
                        cps[:, 0:DH],
                        rc[:],
                    )

            OA = [oa_p.tile([P, 512], F32, tag=f"oa{u}", name=f"oa{u}")
                  for u in range(DJ * 2)]

            pending = None      # (h, n, expt) with scores emitted, ctx not
            for j in range(DJ):
                if j == 4:
                    # early half of the output projection: contraction over
                    # CT[0..2] (ready after head pair 2), staged into SBUF
                    for jo in range(DJ):
                        for sp in range(SLOC // 512):
                            ps = pss.tile([P, 512], F32, tag="s",
                                          name=f"oa{jo}_{sp}")
                            for i in range(3):
                                nc.tensor.matmul(
                                    ps[:],
                                    WO[i][:, P * jo: P * (jo + 1)],
                                    CT[i][:, 512 * sp: 512 * (sp + 1)],
                                    start=(i == 0),
                                    stop=(i == 2),
                                )
                            nc.scalar.activation(
                                OA[jo * 2 + sp][:], ps[:], AF.Copy
                            )
                # Q projection for head pair j
                for sp in range(SLOC // 512):
                    ps = pss.tile([P, 512], F32, tag="s", name=f"q{j}_{sp}")
                    for i in range(DJ):
                        nc.tensor.matmul(
                            ps[:],
                            WQ[i][:, P * j: P * (j + 1)],
                            HS[i][:, W + 512 * sp: W + 512 * (sp + 1)],
                            start=(i == 0),
                            stop=(i == DJ - 1),
                        )
                    nc.scalar.activation(
                        QT[j][:, 512 * sp: 512 * (sp + 1)],
                        ps[:],
                        AF.Identity,
                        bias=bq_sb[:, j: j + 1],
                    )
                # K projection for head pair j (bk cancels in softmax)
                for sp in range(SH // 512):
                    ps = pss.tile([P, 512], F32, tag="s", name=f"k{j}_{sp}")
                    for i in range(DJ):
                        nc.tensor.matmul(
                            ps[:],
                            WK[i][:, P * j: P * (j + 1)],
                            HS[i][:, 512 * sp: 512 * (sp + 1)],
                            start=(i == 0),
                            stop=(i == DJ - 1),
                        )
                    nc.vector.tensor_copy(
                        KT[j][:, 512 * sp: 512 * (sp + 1)], ps[:]
                    )
                # attention units, skew-1 pipelined
                for h in (2 * j, 2 * j + 1):
                    for n in range(NB):
                        expt = emit_scores(h, n)
                        if pending is not None:
                            ph, pn, pexpt = pending
                            emit_ctx(ph, pn, pexpt)
                            if ph % 2 == 1 and pn == NB - 1:
                                pj = ph // 2
                                for n2 in range(NB):
                                    for qh in range(2):
                                        nc.sync.dma_start_transpose(
                                            out=CT[pj][:, W * n2 + P * qh:
                                                       W * n2 + P * (qh + 1)],
                                            in_=CX[2 * n2 + qh][:, P * pj:
                                                                P * (pj + 1)],
                                        )
                        pending = (h, n, expt)
            ph, pn, pexpt = pending
            emit_ctx(ph, pn, pexpt)
            for n2 in range(NB):
                for qh in range(2):
                    nc.sync.dma_start_transpose(
                        out=CT[DJ - 1][:, W * n2 + P * qh: W * n2 + P * (qh + 1)],
                        in_=CX[2 * n2 + qh][:, P * (DJ - 1): P * DJ],
                    )
        hs_stack.close()

        # ---------------- output projection ---------------------------
        with (
            tc.tile_pool(name="ob", bufs=3) as ob_p,
            tc.tile_pool(name="ps3", bufs=4, space="PSUM") as ps3,
        ):
            for j in range(DJ):
                for sp in range(SLOC // 512):
                    ps = ps3.tile([P, 512], F32, tag="ps3")
                    for i in range(3, DJ):
                        nc.tensor.matmul(
                            ps[:],
                            WO[i][:, P * j: P * (j + 1)],
                            CT[i][:, 512 * sp: 512 * (sp + 1)],
                            start=(i == 3),
                            stop=(i == DJ - 1),
                        )
                    osb = ob_p.tile([P, 512], F32, tag="ob")
                    # (late_half + bias) + early_half
                    nc.vector.scalar_tensor_tensor(
                        osb[:], ps[:], boe_sb[:, j: j + 1], OA[j * 2 + sp][:],
                        mybir.AluOpType.add, mybir.AluOpType.add,
                    )
                    nc.sync.dma_start(
                        out=outT[P * j: P * (j + 1), 512 * sp: 512 * (sp + 1)],
                        in_=osb[:],
                    )

    nc.compile()
    return nc


def _band_masks01():
    """Multiplicative band masks, bf16 0/1, [128, 384] each."""
    p = np.arange(P)[:, None]
    c = np.arange(384)[None, :]
    # G0: cols 0:128 = t0 (q = c, valid q <= p); 128:384 = t1 (q = c-128,
    # valid q <= p + 128)
    q0 = np.where(c < 128, c, c - 128)
    v0 = np.where(c < 128, q0 <= p, q0 <= p + 128)
    band0 = v0.astype(ml_dtypes.bfloat16)
    # G2: cols 0:256 = t4 (q = c, valid q >= p); 256:384 = t5 (q = c-128,
    # valid q >= p + 128)
    q2 = np.where(c < 256, c, c - 128)
    v2 = np.where(c < 256, q2 >= p, q2 >= p + 128)
    band2 = v2.astype(ml_dtypes.bfloat16)
    zeros = np.zeros((P, 384), dtype=ml_dtypes.bfloat16)
    return band0, band2, zeros


def _host_prep(hidden_states, attention_mask, Wq, bq, Wk, bk, Wv, bv, Wo, bo):
    """Build per-core input maps. Returns (in_maps, general_mask)."""
    hs = np.asarray(hidden_states, dtype=np.float32)
    am = np.asarray(attention_mask, dtype=np.float32)
    Wq = np.asarray(Wq, dtype=np.float32)
    Wk = np.asarray(Wk, dtype=np.float32)
    Wv = np.asarray(Wv, dtype=np.float32)
    Wo = np.asarray(Wo, dtype=np.float32)
    bq = np.asarray(bq, dtype=np.float32)
    bv = np.asarray(bv, dtype=np.float32)
    bo = np.asarray(bo, dtype=np.float32)

    general = bool(np.any(am != 0.0))
    scale = 1.0 / np.sqrt(np.float32(DH))

    wqT = np.ascontiguousarray(Wq.T * scale).astype(ml_dtypes.bfloat16)
    wkT = np.ascontiguousarray(Wk.T).astype(ml_dtypes.bfloat16)
    wvT = np.ascontiguousarray(Wv.T).astype(ml_dtypes.bfloat16)
    woT = np.ascontiguousarray(Wo.T).astype(ml_dtypes.bfloat16)
    bq_pt = np.ascontiguousarray((bq * scale).reshape(DJ, P).T).astype(np.float32)
    bo_eff = np.ascontiguousarray(
        (bo + Wo @ bv).reshape(DJ, P).T
    ).astype(np.float32)

    band0, band2, zeros = _band_masks01()

    in_maps = []
    for c in range(NCORES):
        bi, g = divmod(c, G)
        lo = SLOC * g - W
        halo = np.zeros((SH, D), dtype=np.float32)
        s0, s1 = max(lo, 0), min(lo + SH, S)
        halo[s0 - lo: s1 - lo] = hs[bi, s0:s1]
        hsT_c = np.ascontiguousarray(halo.T).astype(ml_dtypes.bfloat16)

        m = np.empty((NB, 2, P, 384), dtype=ml_dtypes.bfloat16)
        for n in range(NB):
            m[n, 0] = zeros if (g == 0 and n == 0) else band0
            m[n, 1] = zeros if (g == G - 1 and n == NB - 1) else band2

        in_map = {
            "hsT": hsT_c,
            "wqT": wqT,
            "wkT": wkT,
            "wvT": wvT,
            "woT": woT,
            "bq": bq_pt,
            "boe": bo_eff,
            "masks": m,
        }
        if general:
            # per-key additive bias, constant along queries: [NB, P, 6]
            gb = np.zeros((NB, P, 6), dtype=np.float32)
            p_idx = np.arange(P)[:, None]
            for n in range(NB):
                kglob = SLOC * g - W + W * n + np.arange(6)[None, :] * P + p_idx
                inb = (kglob >= 0) & (kglob < S)
                gb[n] = np.where(inb, -am[bi, np.clip(kglob, 0, S - 1)], 0.0)
            in_map["gbias"] = gb
        in_maps.append(in_map)
    return in_maps, general


def _run(inputs: dict, trace: bool = False):
    """Run the sharded kernel. Returns (full_output, BassKernelResults)."""
    from concourse.bass_utils import run_bass_kernel_spmd

    in_maps, general = _host_prep(**inputs)
    key = ("nc", general)
    if key not in _PROGRAM_CACHE:
        _PROGRAM_CACHE[key] = _build_program(general)
    nc = _PROGRAM_CACHE[key]

    res = run_bass_kernel_spmd(nc, in_maps, list(range(NCORES)), trace=trace)
    out = np.empty((B, S, D), dtype=np.float32)
    for c in range(NCORES):
        bi, g = divmod(c, G)
        out[bi, SLOC * g: SLOC * (g + 1), :] = res.results[c]["outT"].T
    return out, res


def kernel(**inputs) -> np.ndarray:
    out, _ = _run(inputs, trace=False)
    return out


# revision 14
# speedup vs baseline: 1.0258x; 1.0258x over previous
"""Longformer sliding-window self-attention (MBart variant) on 8 TRN2 cores.

Sequence-parallel sharding: core c -> batch c//4, queries
[1024*(c%4), 1024*(c%4+1)). Each core gets a halo'd, transposed,
bf16 slice of hidden states (1536 rows), computes Q/K/V projections,
banded attention, and the output projection, returning a [768, 1024]
fp32 transposed output slice. Host re-assembles [2, 4096, 768].

Design notes (all rewrites exact up to bf16 rounding of the inputs,
measured ~4e-3 max-rel on the fixed problem seed vs the 2e-2 gate):
  - hidden states + weights ship as bf16 (halves HBM traffic); query
    scale folded into Wq/bq, Wk bias cancels in softmax, Wv bias
    commutes through the probs (bo_eff = bo + Wo @ bv).
  - banded scores per (chunk n, head h) are trimmed: key tile 0 only
    covers queries 0:128, tile 5 only 128:256 (rest is out of band) ->
    1280 score columns in three [128,<=512] PSUM tiles.
  - band + sequence-edge masking is a post-exp multiply by a 0/1 bf16
    tile (DVE 2x mode) instead of a -1e9 pre-exp add; a non-zero
    attention_mask (general path) adds its per-key bias pre-exp via
    tensor_scalar (bias is constant along queries = per-partition).
  - context matmuls run TRANSPOSED: lhsT = exp tile [keys, queries],
    rhs = V augmented with a ones column [keys, 65] -> PSUM [queries,
    64 ctx + denominator]. Queries on partitions make the softmax
    denominator a per-partition scalar: one reciprocal [128,1] + one
    tensor_scalar_mul per (n, h, q-half). 128x128 bf16 DMA transposes
    rebuild the [D, seq] layout for the output projection.
  - emission order software-pipelines the whole kernel: V proj, then
    per head-pair j: Q(j), K(j), the 8 attention units (skewed so PE
    streams unit u+1 scores while unit u waits on exp), then that
    pair's DMA transposes. Keeps PE dense (HAM stays warm) and starts
    ScalarE exp work ~50 us earlier.
"""

import numpy as np
import ml_dtypes

# problem shapes (fixed by the task)
B, S, D, H = 2, 4096, 768, 12
DH = D // H            # 64
W = 256                # one-sided window == chunk size b
NEG = -1e9
NCORES = 8
G = 4                  # sequence groups per batch (8 cores / 2 batches)
SLOC = S // G          # 1024 queries per core
SH = SLOC + 2 * W      # 1536 halo'd rows per core
NB = SLOC // W         # 4 chunks per core
P = 128
DJ = D // P            # 6 tiles of 128 over the model dim

_PROGRAM_CACHE: dict = {}

# ctx matmul source columns in the expt slab per q-half: (key tile t, col)
# slab cols: t0(q0:128)->0:128, t1->128:384, t2->384:640, t3->640:896,
#            t4->896:1152, t5(q128:256)->1152:1280
_CTX_QH = [
    [(0, 0), (1, 128), (2, 384), (3, 640), (4, 896)],          # q 0:128
    [(1, 256), (2, 512), (3, 768), (4, 1024), (5, 1152)],      # q 128:256
]


def _build_program(general_mask: bool):
    import concourse.bacc as bacc
    import concourse.mybir as mybir
    import concourse.tile as tile
    from contextlib import ExitStack

    F32 = mybir.dt.float32
    BF = mybir.dt.bfloat16
    AF = mybir.ActivationFunctionType

    nc = bacc.Bacc("TRN2", target_bir_lowering=False, debug=False)

    hsT = nc.dram_tensor("hsT", [D, SH], BF, kind="ExternalInput")
    wqT = nc.dram_tensor("wqT", [D, D], BF, kind="ExternalInput")
    wkT = nc.dram_tensor("wkT", [D, D], BF, kind="ExternalInput")
    wvT = nc.dram_tensor("wvT", [D, D], BF, kind="ExternalInput")
    woT = nc.dram_tensor("woT", [D, D], BF, kind="ExternalInput")
    bq = nc.dram_tensor("bq", [P, DJ], F32, kind="ExternalInput")
    boe = nc.dram_tensor("boe", [P, DJ], F32, kind="ExternalInput")
    masks = nc.dram_tensor("masks", [NB, 2, P, 384], BF, kind="ExternalInput")
    if general_mask:
        gbias = nc.dram_tensor("gbias", [NB, P, 6], F32, kind="ExternalInput")
    outT = nc.dram_tensor("outT", [D, SLOC], F32, kind="ExternalOutput")

    with tile.TileContext(nc) as tc, ExitStack() as stack:
        const = stack.enter_context(tc.tile_pool(name="const", bufs=1))
        qt_p = stack.enter_context(tc.tile_pool(name="qt", bufs=1))
        kt_p = stack.enter_context(tc.tile_pool(name="kt", bufs=1))
        ct_p = stack.enter_context(tc.tile_pool(name="ct", bufs=1))
        va_p = stack.enter_context(tc.tile_pool(name="va", bufs=1))
        w_p = stack.enter_context(tc.tile_pool(name="wts", bufs=1))
        oa_p = stack.enter_context(tc.tile_pool(name="oa", bufs=1))

        # ---- input DMAs, one per tensor, hidden states first ---------
        hs_stack = ExitStack()
        hs_p = hs_stack.enter_context(tc.tile_pool(name="hs", bufs=1))
        hs_all = hs_p.tile([P, DJ * SH], BF, tag="hs")
        hs_view = hs_all.rearrange("p (t c) -> p t c", c=SH)
        hsT_view = hsT.rearrange("(t p) c -> p t c", p=P)
        for ch in range(2):
            nc.sync.dma_start(
                out=hs_view[:, :, SH // 2 * ch: SH // 2 * (ch + 1)],
                in_=hsT_view[:, :, SH // 2 * ch: SH // 2 * (ch + 1)],
            )
        HS = [hs_all[:, SH * i: SH * (i + 1)] for i in range(DJ)]

        wv_all = w_p.tile([P, DJ * D], BF, tag="wv")
        wv_view = wv_all.rearrange("p (t c) -> p t c", c=D)
        wvT_view = wvT.rearrange("(t p) c -> p t c", p=P)
        for ch in range(2):
            nc.sync.dma_start(
                out=wv_view[:, :, 384 * ch: 384 * (ch + 1)],
                in_=wvT_view[:, :, 384 * ch: 384 * (ch + 1)],
            )
        WV = [wv_all[:, D * i: D * (i + 1)] for i in range(DJ)]
        wq_all = w_p.tile([P, DJ * D], BF, tag="wq")
        nc.sync.dma_start(
            out=wq_all.rearrange("p (t c) -> p t c", c=D),
            in_=wqT.rearrange("(t p) c -> p t c", p=P),
        )
        WQ = [wq_all[:, D * i: D * (i + 1)] for i in range(DJ)]
        wk_all = w_p.tile([P, DJ * D], BF, tag="wk")
        nc.sync.dma_start(
            out=wk_all.rearrange("p (t c) -> p t c", c=D),
            in_=wkT.rearrange("(t p) c -> p t c", p=P),
        )
        WK = [wk_all[:, D * i: D * (i + 1)] for i in range(DJ)]

        mask_sb = const.tile([P, NB * 2 * 384], BF, tag="masks")
        nc.sync.dma_start(
            out=mask_sb.rearrange("p (n s c) -> p n s c", s=2, c=384),
            in_=masks.rearrange("n s p c -> p n s c"),
        )
        bq_sb = const.tile([P, DJ], F32, tag="bq")
        nc.sync.dma_start(out=bq_sb[:], in_=bq[:, :])
        boe_sb = const.tile([P, DJ], F32, tag="boe")
        nc.sync.dma_start(out=boe_sb[:], in_=boe[:, :])
        if general_mask:
            gb_sb = const.tile([P, NB * 6], F32, tag="gbias")
            nc.sync.dma_start(
                out=gb_sb.rearrange("p (n c) -> p n c", c=6),
                in_=gbias.rearrange("n p c -> p n c"),
            )
        wo_all = w_p.tile([P, DJ * D], BF, tag="wo")
        nc.sync.dma_start(
            out=wo_all.rearrange("p (t c) -> p t c", c=D),
            in_=woT.rearrange("(t p) c -> p t c", p=P),
        )
        WO = [wo_all[:, D * i: D * (i + 1)] for i in range(DJ)]

        QT = [qt_p.tile([P, SLOC], BF, tag=f"qt{j}", name=f"qt{j}") for j in range(DJ)]
        KT = [kt_p.tile([P, SH], BF, tag=f"kt{j}", name=f"kt{j}") for j in range(DJ)]
        CT = [ct_p.tile([P, SLOC], BF, tag=f"ct{j}", name=f"ct{j}") for j in range(DJ)]
        VA = [va_p.tile([P, H * (DH + 1)], BF, tag=f"va{s}", name=f"va{s}")
              for s in range(SH // P)]
        for st in range(SH // P):
            nc.vector.memset(VA[st][:], 1.0)   # ones columns survive the V copy

        # ---------------- V projection --------------------------------
        with tc.tile_pool(name="ps2", bufs=4, space="PSUM") as ps2:
            for vh in range(2):
                for st in range(SH // P):
                    ps = ps2.tile([P, 384], F32, tag="ps2")
                    for i in range(DJ):
                        nc.tensor.matmul(
                            ps[:],
                            HS[i][:, P * st: P * (st + 1)],
                            WV[i][:, 384 * vh: 384 * (vh + 1)],
                            start=(i == 0),
                            stop=(i == DJ - 1),
                        )
                    view = VA[st].rearrange("p (h e) -> p h e", e=DH + 1)
                    nc.vector.tensor_copy(
                        view[:, 6 * vh: 6 * (vh + 1), 0:DH],
                        ps[:].rearrange("p (h e) -> p h e", e=DH),
                    )

        # ------- interleaved Q/K projections + banded attention -------
        with (
            tc.tile_pool(name="expp", bufs=3) as exp_p,
            tc.tile_pool(name="ctx", bufs=1) as ctx_p,
            tc.tile_pool(name="rcp", bufs=6) as rc_p,
            tc.tile_pool(name="pss", bufs=7, space="PSUM") as pss,
            tc.tile_pool(name="psc", bufs=1, space="PSUM") as psc,
        ):
            CX = [ctx_p.tile([P, H * DH], BF, tag=f"cx{u}", name=f"cx{u}")
                  for u in range(2 * NB)]

            def emit_scores(h, n):
                """Score matmuls + exp + band zeroing for one (head, chunk).
                Returns the bf16 exp slab [128 keys, 1280]."""
                jq, r0 = h // 2, DH * (h % 2)
                kb = W * n
                expt = exp_p.tile([P, 1280], BF, tag="e", name=f"e{n}_{h}")
                segs = [  # (group tiles, exp slab col, mask slot or None)
                    ([(0, 0, 128, 0), (1, 128, 384, 0)], 0, 0),
                    ([(2, 0, 256, 0), (3, 256, 512, 0)], 384, None),
                    ([(4, 0, 256, 0), (5, 256, 384, 128)], 896, 1),
                ]
                for gi, (tiles, ecol, mslot) in enumerate(segs):
                    gp = pss.tile([P, 512], F32, tag="s", name=f"s{gi}_{n}_{h}")
                    width = tiles[-1][2]
                    for t, c0, c1, qlo in tiles:
                        nc.tensor.matmul(
                            gp[:, c0:c1],
                            KT[jq][r0:r0 + DH, kb + P * t: kb + P * (t + 1)],
                            QT[jq][r0:r0 + DH, W * n + qlo: W * n + qlo + (c1 - c0)],
                            start=True,
                            stop=True,
                        )
                    if general_mask:
                        for t, c0, c1, qlo in tiles:
                            nc.vector.tensor_scalar_add(
                                gp[:, c0:c1], gp[:, c0:c1],
                                gb_sb[:, 6 * n + t: 6 * n + t + 1],
                            )
                    nc.scalar.activation(
                        expt[:, ecol: ecol + width], gp[:, 0:width], AF.Exp
                    )
                    if mslot is not None:
                        moff = (n * 2 + mslot) * 384
                        nc.vector.tensor_mul(
                            expt[:, ecol: ecol + 384],
                            expt[:, ecol: ecol + 384],
                            mask_sb[:, moff: moff + 384],
                        )
                return expt

            def emit_ctx(h, n, expt):
                """Transposed context + normalize into the CX slabs."""
                cps = psc.tile([P, 2 * (DH + 1)], F32, tag="c", name=f"c{n}_{h}")
                for qh in range(2):
                    co = (DH + 1) * qh
                    segs = _CTX_QH[qh]
                    for si, (t, c0) in enumerate(segs):
                        nc.tensor.matmul(
                            cps[:, co: co + DH + 1],
                            expt[:, c0: c0 + P],
                            VA[2 * n + t][:, (DH + 1) * h: (DH + 1) * (h + 1)],
                            start=(si == 0),
                            stop=(si == len(segs) - 1),
                        )
                    rc = rc_p.tile([P, 1], F32, tag="rc", name=f"rc{n}_{h}_{qh}")
                    nc.vector.reciprocal(out=rc[:], in_=cps[:, co + DH: co + DH + 1])
                    nc.vector.tensor_scalar_mul(
                        CX[2 * n + qh][:, DH * h: DH * (h + 1)],
                        cps[:, co: co + DH],
                        rc[:],
                    )

            OA = [oa_p.tile([P, 512], F32, tag=f"oa{u}", name=f"oa{u}")
                  for u in range(DJ * 2)]

            pending = []        # (h, n, expt) with scores emitted, ctx not

            def drain_one():
                ph, pn, pexpt = pending.pop(0)
                emit_ctx(ph, pn, pexpt)
                if ph % 2 == 1 and pn == NB - 1:
                    pj = ph // 2
                    for n2 in range(NB):
                        for qh in range(2):
                            nc.sync.dma_start_transpose(
                                out=CT[pj][:, W * n2 + P * qh:
                                           W * n2 + P * (qh + 1)],
                                in_=CX[2 * n2 + qh][:, P * pj: P * (pj + 1)],
                            )

            for j in range(DJ):
                if j == 5:
                    # early 2/3rds of the output projection: contraction over
                    # CT[0..3] (ready after head pair 3), staged into SBUF
                    for jo in range(DJ):
                        for sp in range(SLOC // 512):
                            ps = pss.tile([P, 512], F32, tag="s",
                                          name=f"oa{jo}_{sp}")
                            for i in range(4):
                                nc.tensor.matmul(
                                    ps[:],
                                    WO[i][:, P * jo: P * (jo + 1)],
                                    CT[i][:, 512 * sp: 512 * (sp + 1)],
                                    start=(i == 0),
                                    stop=(i == 3),
                                )
                            nc.scalar.activation(
                                OA[jo * 2 + sp][:], ps[:], AF.Copy
                            )
                # Q projection for head pair j
                for sp in range(SLOC // 512):
                    ps = pss.tile([P, 512], F32, tag="s", name=f"q{j}_{sp}")
                    for i in range(DJ):
                        nc.tensor.matmul(
                            ps[:],
                            WQ[i][:, P * j: P * (j + 1)],
                            HS[i][:, W + 512 * sp: W + 512 * (sp + 1)],
                            start=(i == 0),
                            stop=(i == DJ - 1),
                        )
                    nc.scalar.activation(
                        QT[j][:, 512 * sp: 512 * (sp + 1)],
                        ps[:],
                        AF.Identity,
                        bias=bq_sb[:, j: j + 1],
                    )
                # K projection for head pair j (bk cancels in softmax)
                for sp in range(SH // 512):
                    ps = pss.tile([P, 512], F32, tag="s", name=f"k{j}_{sp}")
                    for i in range(DJ):
                        nc.tensor.matmul(
                            ps[:],
                            WK[i][:, P * j: P * (j + 1)],
                            HS[i][:, 512 * sp: 512 * (sp + 1)],
                            start=(i == 0),
                            stop=(i == DJ - 1),
                        )
                    nc.vector.tensor_copy(
                        KT[j][:, 512 * sp: 512 * (sp + 1)], ps[:]
                    )
                # attention units, skew-1 pipelined
                for h in (2 * j, 2 * j + 1):
                    for n in range(NB):
                        expt = emit_scores(h, n)
                        pending.append((h, n, expt))
                        if len(pending) > 2:
                            drain_one()
            while pending:
                drain_one()
        hs_stack.close()

        # ---------------- output projection ---------------------------
        with (
            tc.tile_pool(name="ob", bufs=3) as ob_p,
            tc.tile_pool(name="ps3", bufs=4, space="PSUM") as ps3,
        ):
            for j in range(DJ):
                osb = ob_p.tile([P, SLOC], F32, tag="ob", name=f"ob{j}")
                for sp in range(SLOC // 512):
                    ps = ps3.tile([P, 512], F32, tag="ps3")
                    for i in range(4, DJ):
                        nc.tensor.matmul(
                            ps[:],
                            WO[i][:, P * j: P * (j + 1)],
                            CT[i][:, 512 * sp: 512 * (sp + 1)],
                            start=(i == 4),
                            stop=(i == DJ - 1),
                        )
                    # (late_half + bias) + early_half
                    nc.vector.scalar_tensor_tensor(
                        osb[:, 512 * sp: 512 * (sp + 1)],
                        ps[:], boe_sb[:, j: j + 1], OA[j * 2 + sp][:],
                        mybir.AluOpType.add, mybir.AluOpType.add,
                    )
                eng = nc.sync if j % 2 == 0 else nc.scalar
                eng.dma_start(
                    out=outT[P * j: P * (j + 1), :], in_=osb[:],
                )

    nc.compile()
    return nc


def _band_masks01():
    """Multiplicative band masks, bf16 0/1, [128, 384] each."""
    p = np.arange(P)[:, None]
    c = np.arange(384)[None, :]
    # G0: cols 0:128 = t0 (q = c, valid q <= p); 128:384 = t1 (q = c-128,
    # valid q <= p + 128)
    q0 = np.where(c < 128, c, c - 128)
    v0 = np.where(c < 128, q0 <= p, q0 <= p + 128)
    band0 = v0.astype(ml_dtypes.bfloat16)
    # G2: cols 0:256 = t4 (q = c, valid q >= p); 256:384 = t5 (q = c-128,
    # valid q >= p + 128)
    q2 = np.where(c < 256, c, c - 128)
    v2 = np.where(c < 256, q2 >= p, q2 >= p + 128)
    band2 = v2.astype(ml_dtypes.bfloat16)
    zeros = np.zeros((P, 384), dtype=ml_dtypes.bfloat16)
    return band0, band2, zeros


def _host_prep(hidden_states, attention_mask, Wq, bq, Wk, bk, Wv, bv, Wo, bo):
    """Build per-core input maps. Returns (in_maps, general_mask)."""
    hs = np.asarray(hidden_states, dtype=np.float32)
    am = np.asarray(attention_mask, dtype=np.float32)
    Wq = np.asarray(Wq, dtype=np.float32)
    Wk = np.asarray(Wk, dtype=np.float32)
    Wv = np.asarray(Wv, dtype=np.float32)
    Wo = np.asarray(Wo, dtype=np.float32)
    bq = np.asarray(bq, dtype=np.float32)
    bv = np.asarray(bv, dtype=np.float32)
    bo = np.asarray(bo, dtype=np.float32)

    general = bool(np.any(am != 0.0))
    scale = 1.0 / np.sqrt(np.float32(DH))

    wqT = np.ascontiguousarray(Wq.T * scale).astype(ml_dtypes.bfloat16)
    wkT = np.ascontiguousarray(Wk.T).astype(ml_dtypes.bfloat16)
    wvT = np.ascontiguousarray(Wv.T).astype(ml_dtypes.bfloat16)
    woT = np.ascontiguousarray(Wo.T).astype(ml_dtypes.bfloat16)
    bq_pt = np.ascontiguousarray((bq * scale).reshape(DJ, P).T).astype(np.float32)
    bo_eff = np.ascontiguousarray(
        (bo + Wo @ bv).reshape(DJ, P).T
    ).astype(np.float32)

    band0, band2, zeros = _band_masks01()

    in_maps = []
    for c in range(NCORES):
        bi, g = divmod(c, G)
        lo = SLOC * g - W
        halo = np.zeros((SH, D), dtype=np.float32)
        s0, s1 = max(lo, 0), min(lo + SH, S)
        halo[s0 - lo: s1 - lo] = hs[bi, s0:s1]
        hsT_c = np.ascontiguousarray(halo.T).astype(ml_dtypes.bfloat16)

        m = np.empty((NB, 2, P, 384), dtype=ml_dtypes.bfloat16)
        for n in range(NB):
            m[n, 0] = zeros if (g == 0 and n == 0) else band0
            m[n, 1] = zeros if (g == G - 1 and n == NB - 1) else band2

        in_map = {
            "hsT": hsT_c,
            "wqT": wqT,
            "wkT": wkT,
            "wvT": wvT,
            "woT": woT,
            "bq": bq_pt,
            "boe": bo_eff,
            "masks": m,
        }
        if general:
            # per-key additive bias, constant along queries: [NB, P, 6]
            gb = np.zeros((NB, P, 6), dtype=np.float32)
            p_idx = np.arange(P)[:, None]
            for n in range(NB):
                kglob = SLOC * g - W + W * n + np.arange(6)[None, :] * P + p_idx
                inb = (kglob >= 0) & (kglob < S)
                gb[n] = np.where(inb, -am[bi, np.clip(kglob, 0, S - 1)], 0.0)
            in_map["gbias"] = gb
        in_maps.append(in_map)
    return in_maps, general


def _run(inputs: dict, trace: bool = False):
    """Run the sharded kernel. Returns (full_output, BassKernelResults)."""
    from concourse.bass_utils import run_bass_kernel_spmd

    in_maps, general = _host_prep(**inputs)
    key = ("nc", general)
    if key not in _PROGRAM_CACHE:
        _PROGRAM_CACHE[key] = _build_program(general)
    nc = _PROGRAM_CACHE[key]

    res = run_bass_kernel_spmd(nc, in_maps, list(range(NCORES)), trace=trace)
    out = np.empty((B, S, D), dtype=np.float32)
    for c in range(NCORES):
        bi, g = divmod(c, G)
        out[bi, SLOC * g: SLOC * (g + 1), :] = res.results[c]["outT"].T
    return out, res


def kernel(**inputs) -> np.ndarray:
    out, _ = _run(inputs, trace=False)
    return out
